# revision 3
# baseline (speedup 1.0000x reference)
"""DPLR transition kernel for Trainium2 (Bass/Tile), SPMD over 8 NeuronCores.

Computes, per (b, h) slice:
    St = Diag(g) S - b k (k^T Diag(g) S) + b k v^T
      = g (.) S + (beta*k) (x) (v - (k*g)^T S)

Sharding: batch (128) split across 8 cores -> 16 batches/core, 32 heads each.

Device-side mapping (per 8-head group, two 4-head halves):
  - mm1 (PE, f32, exact): pu[4,512] = (-k*g)_4^T @ S_4  (head-batched,
    cross-head terms included; only diagonal blocks are meaningful)
  - bridge (DVE): U_bd[4,512] = pu (.) mask_bd  (block-diag mask kills the
    cross terms; PSUM -> SBUF, output rounded to f32r)
  - mm2 (PE, f32r fast path, N=512): po[128,512] = [BK;BK]^T @ [U_bd; V_bd]
    = 4 rank-1 updates beta*k (x) (v - kt) in one matmul via a
    block-diagonal rhs.
  - ACT x4 per half: t = S (.) g_col (exact f32, per-partition scale)
  - DVE: o = t + po ; DMA out

Only the rank-1 correction term goes through the reduced-precision f32r
datapath (11-bit mantissa); the Diag(g) S main term stays exact f32.
Small per-(b,h) operands are pre-laid-out (and pre-rounded where f32r)
on the host so every device op is a full-width tile op.
"""
import sys

sys.path.insert(0, "/opt/trn_rl_repo")

import numpy as np

N_CORES = 8
B, H, K, V = 128, 32, 128, 128
BSH = B // N_CORES   # batches per core
G = 8                # heads per group
NG = H // G          # groups per batch
HALF = 4             # heads per half-group
HCOLS = HALF * V     # 512
AUXW = 2 * HCOLS + 2 * K   # 1280 columns in the aux/rhs tile

_NC_CACHE = {}


def _build_nc():
    if "nc" in _NC_CACHE:
        return _NC_CACHE["nc"]

    from contextlib import ExitStack

    import concourse.bacc as bacc
    import concourse.mybir as mybir
    import concourse.tile as tile

    f32 = mybir.dt.float32
    f32r = mybir.dt.float32r

    nc = bacc.Bacc("TRN2", target_bir_lowering=False)

    state_in = nc.declare_dram_parameter("state_in", [BSH, H, K, V], f32, isOutput=False)
    kgtn = nc.declare_dram_parameter("kgtn", [BSH, K, H], f32, isOutput=False)
    gt = nc.declare_dram_parameter("gt", [BSH, K, H], f32, isOutput=False)
    auxbd = nc.declare_dram_parameter("auxbd", [BSH, NG, G, AUXW], f32r, isOutput=False)
    maskbd = nc.declare_dram_parameter("maskbd", [HALF, HCOLS], f32, isOutput=False)
    out = nc.declare_dram_parameter("out", [BSH, H, K, V], f32, isOutput=True)

    with tile.TileContext(nc) as tc, ExitStack() as ctx:
        s_pool = ctx.enter_context(tc.tile_pool(name="s8", bufs=3))
        t_pool = ctx.enter_context(tc.tile_pool(name="t8", bufs=3))
        o_pool = ctx.enter_context(tc.tile_pool(name="o8", bufs=3))
        aux_pool = ctx.enter_context(tc.tile_pool(name="aux", bufs=4))
        kg_pool = ctx.enter_context(tc.tile_pool(name="kg", bufs=2))
        g_pool = ctx.enter_context(tc.tile_pool(name="gcol", bufs=2))
        const_pool = ctx.enter_context(tc.tile_pool(name="const", bufs=1))
        pu_pool = ctx.enter_context(tc.tile_pool(name="pu", bufs=3, space="PSUM"))
        po_pool = ctx.enter_context(tc.tile_pool(name="po", bufs=3, space="PSUM"))

        mask_t = const_pool.tile([HALF, HCOLS], f32)
        nc.sync.dma_start(mask_t[:], maskbd[:, :])

        for b in range(BSH):
            kg_b = kg_pool.tile([K, H], f32)
            nc.sync.dma_start(kg_b[:], kgtn[b])
            g_b = g_pool.tile([K, H], f32)
            nc.sync.dma_start(g_b[:], gt[b])
            for g in range(NG):
                h0 = g * G
                s8 = s_pool.tile([K, G * V], f32)
                nc.sync.dma_start(
                    s8[:], state_in[b, h0:h0 + G].rearrange("h k v -> k h v")
                )
                # aux tile [8, 1280] f32r:
                #   rows 0:4, cols 0:1024   -> bridge writes U_bd (half 0 / 1)
                #   rows 4:8, cols 0:1024   -> V_bd (block-diag v rows, DMA)
                #   rows 0:8, cols 1024:1280 -> [BK;BK] stacked (DMA)
                aux = aux_pool.tile([G, AUXW], f32r)
                nc.sync.dma_start(aux[:], auxbd[b, g])
                t8 = t_pool.tile([K, G * V], f32)
                o8 = o_pool.tile([K, G * V], f32)
                for hf in range(2):
                    c0 = hf * HCOLS
                    hh = h0 + hf * HALF
                    pu = pu_pool.tile([HALF, HCOLS], f32)
                    nc.tensor.matmul(
                        pu[:],
                        kg_b[:, hh:hh + HALF],
                        s8[:, c0:c0 + HCOLS],
                        start=True, stop=True,
                    )
                    # bridge: mask cross terms, round to f32r into aux rows 0:4
                    nc.vector.tensor_mul(aux[0:HALF, c0:c0 + HCOLS], pu[:], mask_t[:])
                    po = po_pool.tile([K, HCOLS], f32)
                    nc.tensor.matmul(
                        po[:],
                        aux[:, 2 * HCOLS + hf * K: 2 * HCOLS + (hf + 1) * K],
                        aux[:, c0:c0 + HCOLS],
                        start=True, stop=True,
                    )
                    for m in range(HALF):
                        blk = slice(c0 + m * V, c0 + (m + 1) * V)
                        nc.scalar.mul(t8[:, blk], s8[:, blk], g_b[:, hh + m:hh + m + 1])
                    nc.vector.tensor_add(o8[:, c0:c0 + HCOLS], t8[:, c0:c0 + HCOLS], po[:])
                nc.sync.dma_start(
                    out[b, h0:h0 + G].rearrange("h k v -> k h v"), o8[:]
                )

    nc.compile()
    _NC_CACHE["nc"] = nc
    return nc


def _round_f32r(x):
    """Round-to-nearest-even to the f32r format (fp32 with 11-bit mantissa)."""
    u = np.ascontiguousarray(x, np.float32).view(np.uint32)
    u = u + (0x7FF + ((u >> 12) & 1))
    u &= np.uint32(0xFFFFF000)
    return u.view(np.float32)


def _prep_core(keys_c, vals_c, gates_c, beta_c):
    """Host-side layout prep for one core's shard (all small tensors)."""
    kg = keys_c * gates_c                                       # (BSH,H,K)
    kgtn_c = np.ascontiguousarray(-np.swapaxes(kg, 1, 2))       # (BSH,K,H)
    gt_c = np.ascontiguousarray(np.swapaxes(gates_c, 1, 2))     # (BSH,K,H)
    bk = _round_f32r(beta_c * keys_c)                           # (BSH,H,K)
    vr = _round_f32r(vals_c)
    auxbd_c = np.zeros((BSH, NG, G, AUXW), np.float32)
    v5 = vr.reshape(BSH, NG, 2, HALF, V)
    bk5 = bk.reshape(BSH, NG, 2, HALF, K)
    for m in range(HALF):
        # V_bd block-diag rows live on partitions 4..7
        auxbd_c[:, :, HALF + m, V * m:V * (m + 1)] = v5[:, :, 0, m]
        auxbd_c[:, :, HALF + m, HCOLS + V * m:HCOLS + V * (m + 1)] = v5[:, :, 1, m]
    # [BK;BK] stacked on partitions 0..7 for each half
    auxbd_c[:, :, 0:HALF, 2 * HCOLS:2 * HCOLS + K] = bk5[:, :, 0]
    auxbd_c[:, :, HALF:G, 2 * HCOLS:2 * HCOLS + K] = bk5[:, :, 0]
    auxbd_c[:, :, 0:HALF, 2 * HCOLS + K:] = bk5[:, :, 1]
    auxbd_c[:, :, HALF:G, 2 * HCOLS + K:] = bk5[:, :, 1]
    return kgtn_c, gt_c, auxbd_c


def _run(inputs, trace=False, tmpdir=None):
    from concourse.bass_utils import run_bass_kernel_spmd

    state = np.ascontiguousarray(np.asarray(inputs["state"], np.float32))
    keys = np.asarray(inputs["keys"], np.float32)
    values = np.asarray(inputs["values"], np.float32)
    gates = np.asarray(inputs["gates"], np.float32)
    beta = np.asarray(inputs["beta"], np.float32)

    nc = _build_nc()

    mask = np.zeros((HALF, HCOLS), np.float32)
    for m in range(HALF):
        mask[m, V * m:V * (m + 1)] = 1.0

    in_maps = []
    for c in range(N_CORES):
        sl = slice(c * BSH, (c + 1) * BSH)
        kgtn_c, gt_c, auxbd_c = _prep_core(keys[sl], values[sl], gates[sl], beta[sl])
        in_maps.append({
            "state_in": state[sl],
            "kgtn": kgtn_c,
            "gt": gt_c,
            "auxbd": auxbd_c,
            "maskbd": mask,
        })

    res = run_bass_kernel_spmd(nc, in_maps, list(range(N_CORES)),
                               trace=trace, tmpdir=tmpdir)
    full = np.concatenate([res.results[i]["out"] for i in range(N_CORES)], axis=0)
    return full, res


def kernel(**inputs):
    full, _ = _run(inputs, trace=False)
    return full
